# revision 2
# baseline (speedup 1.0000x reference)
"""Distributed Trainium2 kernel for nn_Block_8383776162052 (Chebyshev spectral
graph conv, K=8, V=196608, C=64, random sparse Laplacian 9 nnz/row) on 8
NeuronCores.

Strategy (V-shard, indirect-DMA gather):
- Each core owns 24576 contiguous vertices (original order, no permutation).
- BatchNorm stats via tiny AllReduce; mish on DVE/ACT.
- Per Chebyshev iteration k: the fp16 state table [V,64] is AllGathered in
  three 8192-row slices (pipelined against compute); each core gathers the 9
  neighbor rows per owned vertex straight from DRAM with
  gpsimd.indirect_dma_start (int32 row offsets, CounterMachine descriptor
  generation - ~10x faster than dma_gather's scalar Q7 loop, zero padding).
- Weighted sum + recurrence on DVE in fp16/fp32; final einsum on PE from the
  fp16 state sections via XBAR-transposed loads.
"""
import sys, types, os
sys.path.insert(0, "/opt/trn_rl_repo")
import numpy as np

V = 196608
DEG = 9
C = 64
K = 8
B = 1
EPS = 1e-5
NCORE = 8
VL = V // NCORE          # 24576 rows per core
P = 128
PG = VL // P             # 192 groups (g-major layout: row = g*128 + p)
GCH = 8                  # groups per gather chunk
NCH = PG // GCH          # 24 chunks, 1024 vertices / 9216 descriptors each
THIRD = VL // 3          # 8192-row AllGather slices

_CACHE = {}


def _install_ntff_hook():
    """Shim for missing antenv.axon_hooks (enables trace=True profiling)."""
    import contextlib, ctypes
    if "antenv.axon_hooks" in sys.modules:
        return
    hook_holder = [None]
    mod = types.ModuleType("antenv.axon_hooks")
    mod.get_axon_ntff_profile_hook = lambda: hook_holder[0]
    mod.set_axon_ntff_profile_hook = lambda h: hook_holder.__setitem__(0, h)
    sys.modules["antenv.axon_hooks"] = mod
    so_path = "/opt/axon/libaxon_pjrt.so"
    try:
        lib = ctypes.CDLL(so_path)
        if not hasattr(lib, "axon_start_nrt_profile"):
            return
        lib.axon_start_nrt_profile.argtypes = [ctypes.POINTER(ctypes.c_int64), ctypes.c_size_t]
        lib.axon_start_nrt_profile.restype = ctypes.c_int64
        lib.axon_stop_nrt_profile.argtypes = [ctypes.c_char_p]
        lib.axon_stop_nrt_profile.restype = ctypes.c_int64

        @contextlib.contextmanager
        def _hook(output_dir, device_ids):
            import jax
            jax.devices()
            if device_ids:
                ids = (ctypes.c_int64 * len(device_ids))(*device_ids)
                rc = lib.axon_start_nrt_profile(ids, len(device_ids))
            else:
                rc = lib.axon_start_nrt_profile(None, 0)
            if rc != 0:
                raise RuntimeError(f"axon_start_nrt_profile rc={rc}")
            try:
                yield
            finally:
                n = lib.axon_stop_nrt_profile(str(output_dir).encode())
                print(f"profile: {n} file(s) written to {output_dir}")

        mod.set_axon_ntff_profile_hook(_hook)
    except OSError:
        pass


# ---------------------------------------------------------------------------
# Host-side preprocessing: per-core int32 gather offsets + fp16 edge weights
# ---------------------------------------------------------------------------

def preprocess(lap_rows, lap_cols, lap_vals):
    order = np.argsort(lap_rows, kind="stable")
    cols9 = np.asarray(lap_cols)[order].reshape(V, DEG).astype(np.int64)
    vals9 = np.asarray(lap_vals)[order].reshape(V, DEG).astype(np.float32)

    # Single whole-section AllGather: table row of global vertex u is u itself.
    trow = cols9
    assert trow.min() >= 0 and trow.max() < V
    cores = []
    for s in range(NCORE):
        tr = trow[s * VL:(s + 1) * VL].reshape(PG, P, DEG)    # [g, p, j]
        vv = vals9[s * VL:(s + 1) * VL].reshape(PG, P, DEG)
        iof = np.ascontiguousarray(np.transpose(tr, (1, 0, 2))).reshape(P, PG * DEG)
        w = np.ascontiguousarray(np.transpose(vv, (1, 0, 2))).reshape(P, PG * DEG)
        cores.append(dict(iof=iof.astype(np.int32), w=w.astype(np.float16)))
    return dict(cores=cores)


# ---------------------------------------------------------------------------
# Bass kernel builder (SPMD program, same shapes on all cores)
# ---------------------------------------------------------------------------

def build_kernel(niter=K - 1, dbg=False):
    from concourse import bass, bacc, tile, mybir

    fp32, fp16, i32 = mybir.dt.float32, mybir.dt.float16, mybir.dt.int32
    nc = bacc.Bacc("TRN2", target_bir_lowering=False, debug=False, num_devices=NCORE)

    x_t = nc.dram_tensor("xloc", [VL, C], fp32, kind="ExternalInput")
    iof_t = nc.dram_tensor("iof", [P, PG * DEG], i32, kind="ExternalInput")
    w_t = nc.dram_tensor("wtab", [P, PG * DEG], fp16, kind="ExternalInput")
    gam_t = nc.dram_tensor("gamma", [1, C], fp32, kind="ExternalInput")
    bet_t = nc.dram_tensor("beta", [1, C], fp32, kind="ExternalInput")
    wts_t = nc.dram_tensor("wts", [K, C, C], fp32, kind="ExternalInput")
    bias_t = nc.dram_tensor("bias", [1, C], fp32, kind="ExternalInput")
    out_t = nc.dram_tensor("outT", [C, VL], fp32, kind="ExternalOutput")
    dbg_t = nc.dram_tensor("dbg", [K * VL, C], fp16, kind="ExternalOutput") if dbg else None

    sec16 = [nc.dram_tensor(f"sec16_{k}", [VL, C], fp16, kind="Internal") for k in range(K)]
    # fp32-typed view of the fp16 table bytes: the indirect-DMA offset scaling
    # is only trusted for 4-byte dtypes (production gathers are fp32), so the
    # table is declared [V, 32] fp32 = [V, 64] fp16 bytes.
    tables = [nc.dram_tensor(f"tab{k}", [V, C // 2], fp32, kind="Internal", addr_space="Shared")
              for k in range(K - 1)]
    st_in = nc.dram_tensor("st_in", [1, P], fp32, kind="Internal")
    st_out = nc.dram_tensor("st_out", [1, P], fp32, kind="Internal", addr_space="Shared")

    groups = [list(range(NCORE))]
    CD = GCH * DEG                  # 72 slots per chunk per partition

    def sec_chunk(t, j):
        return t.ap()[j * GCH * P:(j + 1) * GCH * P, :].rearrange("(g p) c -> p g c", p=P)

    with tile.TileContext(nc) as tc:
        with tc.tile_pool(name="res", bufs=1) as res, \
             tc.tile_pool(name="gath", bufs=3) as gth, \
             tc.tile_pool(name="accp", bufs=3) as accp, \
             tc.tile_pool(name="xkp", bufs=3) as xkp, \
             tc.tile_pool(name="sml", bufs=2) as sml, \
             tc.tile_pool(name="cst", bufs=1) as cst, \
             tc.tile_pool(name="ps", bufs=2, space="PSUM") as psp:

            riof = res.tile([P, PG * DEG], i32, tag="riof")
            nc.sync.dma_start(riof[:], iof_t.ap()[:])
            rw = res.tile([P, PG * DEG], fp16, tag="rw")
            nc.sync.dma_start(rw[:], w_t.ap()[:])

            # ---------- phase 0: BN stats + mish ----------
            ph0_cm = tc.tile_pool(name="ph0", bufs=1)
            ph0 = ph0_cm.__enter__()
            xs = ph0.tile([P, PG, C], fp32, tag="xs")
            nc.sync.dma_start(xs[:], x_t.ap()[:].rearrange("(g p) c -> p g c", p=P))
            x2 = ph0.tile([P, PG, C], fp32, tag="x2")
            nc.vector.tensor_tensor(out=x2[:], in0=xs[:], in1=xs[:], op=mybir.AluOpType.mult)
            part = sml.tile([P, 2, C], fp32, tag="part")
            nc.vector.tensor_reduce(
                out=part[:, 0, :], in_=xs[:].rearrange("p g c -> p c g"),
                axis=mybir.AxisListType.X, op=mybir.AluOpType.add)
            nc.vector.tensor_reduce(
                out=part[:, 1, :], in_=x2[:].rearrange("p g c -> p c g"),
                axis=mybir.AxisListType.X, op=mybir.AluOpType.add)
            ones = cst.tile([P, 1], fp32, tag="ones")
            nc.vector.memset(ones[:], 1.0)
            ps_sum = psp.tile([1, 2 * C], fp32, tag="ps_small")
            nc.tensor.matmul(out=ps_sum[:], lhsT=ones[:], rhs=part[:].rearrange("p a c -> p (a c)"),
                             start=True, stop=True)
            sums = sml.tile([1, 2 * C], fp32, tag="sums")
            nc.vector.tensor_copy(out=sums[:], in_=ps_sum[:])
            nc.sync.dma_start(st_in.ap()[:], sums[:])
            nc.gpsimd.collective_compute(
                "AllReduce", mybir.AluOpType.add,
                replica_groups=groups,
                ins=[st_in.ap().opt()], outs=[st_out.ap().opt()])
            gsums = sml.tile([1, 2 * C], fp32, tag="gsums")
            nc.sync.dma_start(gsums[:], st_out.ap()[:])
            gam = sml.tile([1, C], fp32, tag="gam")
            bet = sml.tile([1, C], fp32, tag="bet")
            nc.sync.dma_start(gam[:], gam_t.ap()[:])
            nc.sync.dma_start(bet[:], bet_t.ap()[:])
            mean = sml.tile([1, C], fp32, tag="mean")
            nc.vector.tensor_scalar_mul(mean[:], gsums[:, :C], 1.0 / (B * V))
            ex2 = sml.tile([1, C], fp32, tag="ex2")
            nc.vector.tensor_scalar_mul(ex2[:], gsums[:, C:], 1.0 / (B * V))
            m2 = sml.tile([1, C], fp32, tag="m2")
            nc.vector.tensor_tensor(out=m2[:], in0=mean[:], in1=mean[:], op=mybir.AluOpType.mult)
            var = sml.tile([1, C], fp32, tag="var")
            nc.vector.tensor_tensor(out=var[:], in0=ex2[:], in1=m2[:], op=mybir.AluOpType.subtract)
            epsT = cst.tile([1, 1], fp32, tag="epsT")
            nc.vector.memset(epsT[:], EPS)
            sd = sml.tile([1, C], fp32, tag="sd")
            nc.scalar.activation(sd[:], var[:], mybir.ActivationFunctionType.Sqrt, bias=epsT[:])
            rstd = sml.tile([1, C], fp32, tag="rstd")
            nc.vector.reciprocal(rstd[:], sd[:])
            Av = sml.tile([1, C], fp32, tag="Av")
            nc.vector.tensor_tensor(out=Av[:], in0=rstd[:], in1=gam[:], op=mybir.AluOpType.mult)
            mA = sml.tile([1, C], fp32, tag="mA")
            nc.vector.tensor_tensor(out=mA[:], in0=mean[:], in1=Av[:], op=mybir.AluOpType.mult)
            Bv = sml.tile([1, C], fp32, tag="Bv")
            nc.vector.tensor_tensor(out=Bv[:], in0=bet[:], in1=mA[:], op=mybir.AluOpType.subtract)
            AB = sml.tile([1, 2 * C], fp32, tag="AB")
            nc.vector.tensor_copy(out=AB[:, :C], in_=Av[:])
            nc.vector.tensor_copy(out=AB[:, C:], in_=Bv[:])
            one1 = cst.tile([1, P], fp32, tag="one1")
            nc.vector.memset(one1[:], 1.0)
            ps_ab = psp.tile([P, 2 * C], fp32, tag="ps_small")
            nc.tensor.matmul(out=ps_ab[:], lhsT=one1[:], rhs=AB[:], start=True, stop=True)
            ABb = cst.tile([P, 2 * C], fp32, tag="ABb")
            nc.vector.tensor_copy(out=ABb[:], in_=ps_ab[:])

            # h = mish(x*A + B); reuse x2 as scratch
            nc.vector.tensor_tensor(
                out=x2[:], in0=xs[:],
                in1=ABb[:, :C].unsqueeze(1).to_broadcast([P, PG, C]),
                op=mybir.AluOpType.mult)
            nc.vector.tensor_tensor(
                out=x2[:], in0=x2[:],
                in1=ABb[:, C:].unsqueeze(1).to_broadcast([P, PG, C]),
                op=mybir.AluOpType.add)
            zeroP = cst.tile([P, 1], fp32, tag="zeroP")
            nc.vector.memset(zeroP[:], 0.0)
            x0 = ph0.tile([P, PG, C], fp32, tag="xs")   # share slot with xs
            HG = PG // 2
            for h in range(2):
                sl = slice(h * HG, (h + 1) * HG)
                u = ph0.tile([P, HG, C], fp32, tag="u")
                nc.scalar.activation(u[:], x2[:, sl, :], mybir.ActivationFunctionType.Exp, bias=zeroP[:])
                nc.vector.scalar_tensor_tensor(
                    out=u[:], in0=u[:], scalar=2.0, in1=u[:],
                    op0=mybir.AluOpType.add, op1=mybir.AluOpType.mult)   # (u+2)*u
                nc.vector.tensor_scalar_add(u[:], u[:], 2.0)             # u(u+2)+2
                nc.vector.reciprocal(u[:], u[:])
                nc.vector.tensor_scalar(out=u[:], in0=u[:], scalar1=-2.0, scalar2=1.0,
                                        op0=mybir.AluOpType.mult, op1=mybir.AluOpType.add)
                nc.vector.tensor_tensor(out=x0[:, sl, :], in0=x2[:, sl, :], in1=u[:],
                                        op=mybir.AluOpType.mult)
            h16 = ph0.tile([P, PG, C], fp16, tag="x2")   # share slot with x2
            nc.scalar.activation(h16[:], x0[:], mybir.ActivationFunctionType.Copy)
            nc.sync.dma_start(sec16[0].ap()[:].rearrange("(g p) c -> p g c", p=P), h16[:])
            nc.gpsimd.collective_compute(
                "AllGather", mybir.AluOpType.bypass,
                replica_groups=groups,
                ins=[sec16[0].ap().bitcast(fp32).opt()], outs=[tables[0].ap().opt()])
            ph0_cm.__exit__(None, None, None)

            # ---------- Chebyshev iterations ----------
            SPC = 8                       # slots per single-packet gather call
            for k in range(1, 1 + niter):
                for j in range(NCH):
                    gsb = gth.tile([P, CD, C // 2], fp32, tag="gsb")
                    for o in range(CD):
                        # HW indirect DMA consumes exactly one index per
                        # partition per instruction (verified on-device)
                        inst = nc.gpsimd.indirect_dma_start(
                            out=gsb[:, o, :], out_offset=None,
                            in_=tables[k - 1].ap()[:],
                            in_offset=bass.IndirectOffsetOnAxis(
                                ap=riof[:, j * CD + o:j * CD + o + 1],
                                axis=0),
                        )
                        inst.single_packet = True
                    gv = gsb[:].bitcast(fp16)
                    nc.vector.tensor_tensor(
                        out=gv, in0=gv,
                        in1=rw[:, j * CD:(j + 1) * CD].unsqueeze(2).to_broadcast([P, CD, C]),
                        op=mybir.AluOpType.mult)
                    acc = accp.tile([P, GCH, C], fp32, tag="acc")
                    nc.vector.tensor_reduce(
                        out=acc[:], in_=gv.rearrange("p (g s) c -> p g c s", s=DEG),
                        axis=mybir.AxisListType.X, op=mybir.AluOpType.add)
                    xk16 = xkp.tile([P, GCH, C], fp16, tag="xk16")
                    if k == 1:
                        nc.scalar.activation(xk16[:], acc[:], mybir.ActivationFunctionType.Copy)
                    else:
                        xk2 = xkp.tile([P, GCH, C], fp16, tag="xk2")
                        nc.sync.dma_start(xk2[:], sec_chunk(sec16[k - 2], j))
                        xk2f = xkp.tile([P, GCH, C], fp32, tag="xk2f")
                        nc.vector.tensor_copy(out=xk2f[:], in_=xk2[:])
                        xkf = accp.tile([P, GCH, C], fp32, tag="xkf")
                        nc.vector.scalar_tensor_tensor(
                            out=xkf[:], in0=acc[:], scalar=2.0, in1=xk2f[:],
                            op0=mybir.AluOpType.mult, op1=mybir.AluOpType.subtract)
                        nc.scalar.activation(xk16[:], xkf[:], mybir.ActivationFunctionType.Copy)
                    nc.sync.dma_start(sec_chunk(sec16[k], j), xk16[:])
                if k < K - 1:
                    nc.gpsimd.collective_compute(
                        "AllGather", mybir.AluOpType.bypass,
                        replica_groups=groups,
                        ins=[sec16[k].ap().bitcast(fp32).opt()], outs=[tables[k].ap().opt()])

            # ---------- optional state dump ----------
            if dbg:
                for k in range(K):
                    for j in range(NCH):
                        dt_ = xkp.tile([P, GCH, C], fp16, tag="dbgt")
                        nc.sync.dma_start(dt_[:], sec_chunk(sec16[k], j))
                        nc.sync.dma_start(
                            dbg_t.ap()[k * VL + j * GCH * P:k * VL + (j + 1) * GCH * P, :]
                            .rearrange("(g p) c -> p g c", p=P), dt_[:])

            # ---------- output pass ----------
            wts = cst.tile([2 * C, K, C], fp32, tag="wts")
            nc.sync.dma_start(wts[:C], wts_t.ap()[:].rearrange("k i o -> i k o"))
            nc.sync.dma_start(wts[C:], wts_t.ap()[:].rearrange("k i o -> i k o"))
            wts16 = cst.tile([2 * C, K, C], fp16, tag="wts16")
            nc.scalar.activation(wts16[:], wts[:], mybir.ActivationFunctionType.Copy)
            bias_sb = sml.tile([1, C], fp32, tag="biasv")
            nc.sync.dma_start(bias_sb[:], bias_t.ap()[:])
            ps_b = psp.tile([C, 1], fp32, tag="ps_small")
            nc.tensor.matmul(out=ps_b[:], lhsT=bias_sb[:], rhs=one1[:, :1],
                             start=True, stop=True)
            biasT = cst.tile([C, 1], fp32, tag="biasT")
            nc.vector.tensor_copy(out=biasT[:], in_=ps_b[:])

            NV = 512
            with tc.tile_pool(name="xtp", bufs=K + 1) as xtp, \
                 tc.tile_pool(name="ots", bufs=3) as otp:
                for t in range(VL // NV):
                    pse = psp.tile([C, NV // 2], fp32, tag="pse")
                    pso = psp.tile([C, NV // 2], fp32, tag="pso")
                    xts = []
                    for k in range(K):
                        xt = xtp.tile([P, NV // 2], fp16, tag="xt")
                        srcap = sec16[k].ap()[:].rearrange("(a b) c -> a (b c)", b=2)[
                            t * (NV // 2):(t + 1) * (NV // 2), :]
                        nc.sync.dma_start(xt[:], srcap, transpose=True)
                        xts.append(xt)
                    for k in range(K):
                        nc.tensor.matmul(out=pse[:], lhsT=wts16[:C, k, :],
                                         rhs=xts[k][:C, :], start=(k == 0), stop=(k == K - 1))
                    for k in range(K):
                        nc.tensor.matmul(out=pso[:], lhsT=wts16[C:, k, :],
                                         rhs=xts[k][C:, :], start=(k == 0), stop=(k == K - 1))
                    ot = otp.tile([C, NV], fp32, tag="ot")
                    ov = ot[:].rearrange("c (a b) -> c a b", b=2)
                    nc.scalar.activation(ov[:, :, 0], pse[:],
                                         mybir.ActivationFunctionType.Identity, bias=biasT[:])
                    nc.scalar.activation(ov[:, :, 1], pso[:],
                                         mybir.ActivationFunctionType.Identity, bias=biasT[:])
                    nc.sync.dma_start(out_t.ap()[:, t * NV:(t + 1) * NV], ot[:])

    nc.compile()
    return nc


# ---------------------------------------------------------------------------
# Public entry point
# ---------------------------------------------------------------------------

def kernel(x, lap_rows, lap_cols, lap_vals, gamma, beta, weight, bias, _trace=False):
    _install_ntff_hook()
    from concourse.bass_utils import run_bass_kernel_spmd

    lap_rows = np.asarray(lap_rows)
    lap_cols = np.asarray(lap_cols)
    lap_vals = np.asarray(lap_vals, np.float32)
    x = np.asarray(x, np.float32)
    gamma = np.asarray(gamma, np.float32).reshape(1, C)
    beta = np.asarray(beta, np.float32).reshape(1, C)
    weight = np.asarray(weight, np.float32)
    bias = np.asarray(bias, np.float32).reshape(1, C)

    dbg = bool(int(os.environ.get("KDBG", "0")))
    key = (int(lap_cols[0]), int(lap_cols[-1]), int(lap_rows[7]), dbg)
    if "meta" not in _CACHE or _CACHE.get("key") != key:
        meta = preprocess(lap_rows, lap_cols, lap_vals)
        nc = build_kernel(dbg=dbg)
        _CACHE.update(meta=meta, nc=nc, key=key)
    meta, nc = _CACHE["meta"], _CACHE["nc"]

    in_maps = []
    for s in range(NCORE):
        cd = meta["cores"][s]
        in_maps.append({
            "xloc": np.ascontiguousarray(x[0, s * VL:(s + 1) * VL, :]),
            "iof": cd["iof"], "wtab": cd["w"],
            "gamma": gamma, "beta": beta, "wts": weight, "bias": bias,
        })
    res = run_bass_kernel_spmd(nc, in_maps, core_ids=list(range(NCORE)), trace=_trace)
    kernel.last_res = res
    out = np.empty((1, V, C), np.float32)
    for s in range(NCORE):
        out[0, s * VL:(s + 1) * VL, :] = res.results[s]["outT"].T
    kernel.last_exec_time_ns = res.exec_time_ns
    return out
